# revision 47
# baseline (speedup 1.0000x reference)
# Trainium2 Bass kernel for nn_Bert_79817672229402 (DeBERTa-style disentangled
# attention transformer). Batch-parallel over 8 NeuronCores (B=8, one batch
# element per core). All shapes hardcoded per the problem spec.
#
# Restructured vs the original baseline for pipeline occupancy:
#   - weights stream ONCE per layer (wv was re-streamed 4x), wv in bf16
#   - rel-position skew handled by diagonal APs fed straight into the
#     score-inject matmuls (no SBUF->SBUF skew DMAs at all)
#   - expansion width 640 per q-tile window (was 1023)
#   - ACT function-set batching: gelu only in the value phase, exp only in
#     the scores phase (act-table reloads cost 1.28us each)
#   - PSUM: pool A = 4 x 1-bank [128,512] f32 (GEMM accums / scores / ctx),
#     pool E = 2 x 2-bank [128,1024] f32 (transposes / tables / expansion)
import math
import os

import numpy as np

S, B, H, NH, I, L, V, BK, MP = 512, 8, 768, 12, 2304, 4, 16384, 32, 512
DH = H // NH          # 64
DV = I // NH          # 192
EPS = 1e-7
SCALE = 1.0 / math.sqrt(3 * DH)
NT = S // 128         # 4 token tiles
NCH = H // 128        # 6 channel tiles
NCI = I // 128        # 18 ctx channel tiles
W = 2 * S - 1         # 1023 expansion width
WPAD = 1024           # padded table width (col 1023 is zero, never consumed)
NJ = 2 * BK - 1       # 63 relative buckets
EW = 640              # per-q-tile expansion window width
QPITCH = NT * EW      # 2560 flat pitch of qr/kr tiles

LAST_RESULT = [None]


def _build_program(nc, mybir, bass, tile, make_identity):
    f32 = mybir.dt.float32
    f32r = mybir.dt.float32r
    bf16 = mybir.dt.bfloat16
    AF = mybir.ActivationFunctionType

    # ---------------- DRAM I/O ----------------
    d_x0 = nc.dram_tensor("x0", [S, H], f32, kind="ExternalInput")
    d_mb = nc.dram_tensor("maskbias", [128, NT], f32, kind="ExternalInput")
    d_rel = nc.dram_tensor("rel_emb", [NJ, H], f32, kind="ExternalInput")
    d_relw = nc.dram_tensor("rel_w", [NJ, H], f32, kind="ExternalInput")
    d_relb = nc.dram_tensor("rel_b", [NJ, H], f32, kind="ExternalInput")
    d_s2 = nc.dram_tensor("s2", [NJ, WPAD], bf16, kind="ExternalInput")
    d_s3 = nc.dram_tensor("s3", [NJ, WPAD], bf16, kind="ExternalInput")
    d_wqk = nc.dram_tensor("wqkT", [L, H + 1, 2 * H], bf16, kind="ExternalInput")
    d_wv = nc.dram_tensor("wvT", [L, H, 2 * I], bf16, kind="ExternalInput")
    d_wo = nc.dram_tensor("woT", [L, I, H], bf16, kind="ExternalInput")
    d_sig = nc.dram_tensor("sig", [L, 128, I], bf16, kind="ExternalInput")
    d_bias = nc.dram_tensor("biasT", [L, 128, 12], f32, kind="ExternalInput")
    d_ident = nc.dram_tensor("ident", [128, 128], f32r, kind="ExternalInput")
    d_out = nc.dram_tensor("out", [S, H], f32, kind="ExternalOutput")

    from contextlib import ExitStack

    tc = tile.TileContext(nc)

    with tc, ExitStack() as es:
        pools = {}

        def pool(name, bufs, space="SBUF"):
            if name not in pools:
                pools[name] = es.enter_context(
                    tc.tile_pool(name=name, bufs=bufs, space=space))
            return pools[name]

        const = pool("const", 1)
        xp = pool("xp", 1)
        htp = pool("htp", 1)
        qkp = pool("qkp", 1)
        qkbp = pool("qkbp", 2)
        vgp = pool("vgp", 1)
        posp = pool("posp", 1)
        expp = pool("expp", 1)     # kpe/qpe tables
        qrp = pool("qrp", 1)       # qr/kr diagonal tiles
        probp = pool("probp", 1)
        wstream = pool("wstream", 5)
        wstream2 = pool("wstream2", 5)
        sigp = pool("sigp", 1)
        small = pool("small", 4)
        tmpp = pool("tmpp", 4)
        ctxtp = pool("ctxtp", 3)
        # PSUM: 8 banks total.  A: 4 x [128,512] f32 (1 bank each)
        #                       E: 2 x [128,1024] f32 (2 banks each)
        ps_a = pool("ps_a", 4, space="PSUM")
        ps_e = pool("ps_e", 2, space="PSUM")

        # ---------------- constants ----------------
        ident_bf = const.tile([128, 128], bf16)
        make_identity(nc, ident_bf)
        ident_fr = const.tile([128, 128], f32r)
        nc.sync.dma_start(ident_fr, d_ident[:])
        mb_sb = const.tile([128, NT], f32)
        nc.sync.dma_start(mb_sb, d_mb[:])
        s2_sb = const.tile([NJ, WPAD], bf16)
        nc.sync.dma_start(s2_sb, d_s2[:])
        s3_sb = const.tile([NJ, WPAD], bf16)
        nc.sync.dma_start(s3_sb, d_s3[:])

        # ---------------- LN helpers (token-major) ----------------
        def emit_stats(x_aps, P, D, tag="ln_stats"):
            n = len(x_aps)
            nsub = D // 256
            stats = tmpp.tile([128, n, nsub, 6], f32, tag=tag, name="ln_stats",
                              bufs=2)
            for t, x_ap in enumerate(x_aps):
                if x_ap is None:
                    continue
                for i in range(nsub):
                    nc.vector.bn_stats(stats[:P, t, i, :],
                                       x_ap[:, i * 256:(i + 1) * 256])
            return stats

        def add_stats(stats, t, x_ap, P, D):
            nsub = D // 256
            for i in range(nsub):
                nc.vector.bn_stats(stats[:P, t, i, :],
                                   x_ap[:, i * 256:(i + 1) * 256])

        def finish_ln(stats, P, n):
            mv = tmpp.tile([128, n, 2], f32, tag="ln_mv", name="ln_mv")
            for t in range(n):
                nc.vector.bn_aggr(mv[:P, t, :], stats[:P, t])
            eps_t = tmpp.tile([128, 1], f32, tag="ln_eps", name="ln_eps")
            nc.vector.memset(eps_t[:P], EPS)
            rstd = tmpp.tile([128, n], f32, tag="ln_rstd", name="ln_rstd")
            var_ap = bass.AP(mv.tensor, mv.offset + 1, [[2 * n, P], [2, n]])
            nc.scalar.activation(rstd[:P], var_ap, AF.Sqrt, bias=eps_t[:P], scale=1.0)
            nc.vector.reciprocal(rstd[:P], rstd[:P])
            negmr = tmpp.tile([128, n], f32, tag="ln_negmr", name="ln_negmr")
            mean_ap = bass.AP(mv.tensor, mv.offset, [[2 * n, P], [2, n]])
            nc.vector.tensor_mul(negmr[:P], mean_ap, rstd[:P])
            nc.vector.tensor_scalar_mul(negmr[:P], negmr[:P], -1.0)
            return negmr, rstd

        def ln_stats(x_aps, P, D, tag="ln_stats"):
            return finish_ln(emit_stats(x_aps, P, D, tag=tag), P, len(x_aps))

        # ---------------- initial x = LN(word_emb[ids]) in place ----------------
        x_tiles = []
        for t in range(NT):
            xt = xp.tile([128, H], f32, tag=f"x{t}", name=f"x{t}")
            x_tiles.append(xt)
            nc.sync.dma_start(xt, d_x0[t * 128:(t + 1) * 128, :])
        negmr0, rstd0 = ln_stats([x[:] for x in x_tiles], 128, H)
        for t in range(NT):
            nc.scalar.activation(x_tiles[t][:], x_tiles[t][:], AF.Identity,
                                 bias=negmr0[:, t:t + 1], scale=rstd0[:, t:t + 1])
        h_stats = emit_stats([x[:] for x in x_tiles], 128, H, tag="h_stats")
        x_bf = [htp.tile([128, H], bf16, tag=f"xbf{t}", name=f"xbf{t}")
                for t in range(NT)]
        for t in range(NT):
            nc.vector.tensor_copy(x_bf[t][:], x_tiles[t][:])

        # ---------------- rel path: rel_ln = LN(rel_emb)*w + b; relT ----------------
        relt_stage = tmpp.tile([NJ, H], f32, tag="h2", name="relt_stage",
                                bufs=1)
        nc.sync.dma_start(relt_stage, d_rel[:])
        negmr_r, rstd_r = ln_stats([relt_stage[:]], NJ, H)
        nc.scalar.activation(relt_stage[:], relt_stage[:], AF.Identity,
                             bias=negmr_r[:NJ, 0:1], scale=rstd_r[:NJ, 0:1])
        relw_t = tmpp.tile([NJ, H], f32, tag="h0", name="relw", bufs=1)
        nc.sync.dma_start(relw_t, d_relw[:])
        nc.vector.tensor_mul(relt_stage[:], relt_stage[:], relw_t[:])
        relb_t = tmpp.tile([NJ, H], f32, tag="h1", name="relb", bufs=1)
        nc.sync.dma_start(relb_t, d_relb[:])
        rel_fin = tmpp.tile([NJ, H], bf16, tag="h3", name="rel_fin",
                              bufs=1)
        nc.vector.tensor_add(rel_fin[:], relt_stage[:], relb_t[:])
        # transpose -> relT [128, NCH, 64] bf16 (cols 0:63 valid)
        relT = const.tile([128, NCH, 64], bf16)
        for c in range(NCH):
            pt = ps_e.tile([128, 1024], bf16, tag="E", name="tr_ps")
            nc.tensor.transpose(pt[:, 0:63], rel_fin[:, c * 128:(c + 1) * 128],
                                ident_bf[:NJ, :NJ])
            nc.vector.tensor_copy(relT[:, c, 0:63], pt[:, 0:63])

        # ================ layers ================
        for l in range(L):
            # ---- h = LN(x) folded into bf16 affine-transpose matmuls ----
            negmr, rstd = finish_ln(h_stats, 128, NT)
            for t in range(NT):
                nc.vector.tensor_scalar_mul(dgh_r[:, t, :], ident_bf[:],
                                            rstd[:, t:t + 1])
                nc.vector.tensor_scalar_mul(dgh_n[:, t, :], ident_bf[:],
                                            negmr[:, t:t + 1])
            hT_bf = []
            for c in range(NCH):
                pt = ps_e.tile([128, 1024], f32, tag="E", name="htr_ps")
                for t in range(NT):
                    nc.tensor.matmul(pt[:, t * 128:(t + 1) * 128],
                                     x_bf[t][:, c * 128:(c + 1) * 128],
                                     dgh_r[:, t, :], start=True, stop=True,
                                     skip_group_check=(t > 0))
                    nc.tensor.matmul(pt[:, t * 128:(t + 1) * 128],
                                     ones_bf[:], dgh_n[:, t, :],
                                     start=False, stop=True,
                                     skip_group_check=True)
                hcb = htp.tile([128, 512], bf16, tag=f"hTb{c}", name=f"hTb{c}")
                nc.vector.tensor_copy(hcb[:], pt[:, 0:512])
                hT_bf.append(hcb)

            # ---- bias column tile [128, 12] ----
            bias_sb = tmpp.tile([128, 12], f32, tag="bias", name="bias", bufs=2)
            nc.sync.dma_start(bias_sb, d_bias[l])

            # ---- qkT (12 m-tiles [128,512] f32r) + pos projection, fused ----
            qkT = [None] * 12
            pos_sb = posp.tile([NJ, 2 * H], bf16, tag="pos", name="pos")
            for n in range(3):
                acc = [ps_a.tile([128, 512], f32, tag="A", name="qk_ps")
                       for _ in range(4)]
                psp = ps_e.tile([128, 1024], f32, tag="E", name="pos_ps")
                wchunk = wstream.tile([128, NCH, 512], bf16, tag="wqk_l",
                                      name="wqk_l", bufs=2)
                _w0 = d_wqk[l]
                wsrc = bass.AP(_w0.tensor, _w0.offset + n * 512,
                               [[2 * H, 128], [128 * 2 * H, NCH], [1, 512]])
                nc.sync.dma_start(wchunk[:], wsrc)
                for i in range(4):
                    for c in range(NCH):
                        nc.tensor.matmul(acc[i],
                                         wchunk[:, c, i * 128:(i + 1) * 128],
                                         hT_bf[c][:], start=(c == 0),
                                         stop=(c == NCH - 1))
                for c in range(NCH):
                    nc.tensor.matmul(psp[:63, 0:512], relT[:, c, 0:63],
                                     wchunk[:, c, :],
                                     start=(c == 0), stop=(c == NCH - 1))
                for i in range(4):
                    m = n * 4 + i
                    qt_t = qkp.tile([128, 512], bf16, tag=f"qkT{m}", name=f"qkT{m}")
                    nc.scalar.activation(qt_t[:], acc[i], AF.Identity,
                                         bias=bias_sb[:, m:m + 1], scale=1.0)
                    qkT[m] = qt_t
                nc.vector.tensor_copy(pos_sb[:, n * 512:(n + 1) * 512], psp[:NJ, 0:512])

            # ---- sigmoid(l_skip) replicated, bf16 ----
            sig_sb = sigp.tile([128, I], bf16, tag="sig", name="sig")
            nc.sync.dma_start(sig_sb, d_sig[l])

            pend = emit_tables_expansion_holder[0](0)

            # ---- value/gate GEMM (wv bf16, streamed once) ----
            # v_aug token-major [128, NH, DV+1] with ones col; gate/vsg [128, I]
            v_aug = [vgp.tile([128, NH, DV + 1], bf16, tag=f"vaug{t}", name=f"vaug{t}")
                     for t in range(NT)]
            gate = [vgp.tile([128, I], bf16, tag=f"gate{t}", name=f"gate{t}")
                    for t in range(NT)]
            vsg = [vgp.tile([128, I], bf16, tag=f"vsg{t}", name=f"vsg{t}")
                   for t in range(NT)]
            if l == 0:
                for t in range(NT):
                    nc.vector.memset(v_aug[t][:, :, DV:DV + 1], 1.0)
            NCHUNK = 384

            def emit_value_chunk(n):
                acc = [ps_a.tile([128, 512], f32, tag="A", name="v_ps")
                       for _ in range(NT)]
                wchunk = wstream2.tile([128, NCH, NCHUNK], bf16, tag="wv_l",
                                       name="wv_l", bufs=3)
                _w0 = d_wv[l]
                wsrc = bass.AP(_w0.tensor, _w0.offset + n * NCHUNK,
                               [[2 * I, 128], [128 * 2 * I, NCH], [1, NCHUNK]])
                nc.sync.dma_start(wchunk[:], wsrc)
                for t in range(NT):
                    for c in range(NCH):
                        nc.tensor.matmul(acc[t][:, 0:NCHUNK],
                                         hT_bf[c][:, t * 128:(t + 1) * 128],
                                         wchunk[:, c, :], start=(c == 0),
                                         stop=(c == NCH - 1))
                if n < 6:      # value chunk: heads 2n, 2n+1
                    for t in range(NT):
                        dst = bass.AP(v_aug[t].tensor,
                                      v_aug[t].offset + 2 * n * (DV + 1),
                                      [[NH * (DV + 1), 128], [DV + 1, 2], [1, DV]])
                        nc.vector.tensor_copy(dst, acc[t][:, 0:NCHUNK])
                        # gelu from the bf16 copy (SBUF) so ACT never reads the
                        # same PSUM bank DVE is reading
                        vdst = bass.AP(vsg[t].tensor,
                                       vsg[t].offset + n * NCHUNK,
                                       [[I, 128], [DV, 2], [1, DV]])
                        nc.scalar.activation(vdst, dst, AF.Gelu,
                                             bias=0.0, scale=1.0)
                else:          # gate chunk
                    gn = n - 6
                    for t in range(NT):
                        nc.scalar.activation(
                            gate[t][:, gn * NCHUNK:(gn + 1) * NCHUNK],
                            acc[t][:, 0:NCHUNK], AF.Gelu, bias=0.0, scale=1.0)

            for n in range(6):        # value half
                emit_value_chunk(n)
            for t in range(NT):
                nc.gpsimd.tensor_mul(vsg[t][:], vsg[t][:], sig_sb[:])
            pr0 = emit_scores(0, *pend)
            pend = emit_tables_expansion(1)
            for n in range(6, 12):    # gate half
                emit_value_chunk(n)

            ctx_stats = tmpp.tile([128, NT, NH, 6], f32, tag="ctx_stats",
                                  name="ctx_stats", bufs=2)
            emit_ctx(0, pr0)
            for hpi in range(1, NH // 2):
                # expansion(p+1) first: its evac->skew chain hides under the
                # scores(p) matmuls instead of the short ctx(p) tail
                nxt = (emit_tables_expansion(hpi + 1)
                       if hpi + 1 < NH // 2 else None)
                pr = emit_scores(hpi, *pend)
                pend = nxt
                emit_ctx(hpi, pr)

            # ---- ctx-LN folded into the Wo transpose matmuls via diagonals;
            #      finished per token-tile pair so Wo pass 0 starts early ----
            def finish_ctx_ln(ts):
                n = len(ts)
                mv = tmpp.tile([128, NT, 2], f32, tag="cmv", name="cmv", bufs=2)
                for t in ts:
                    nc.vector.bn_aggr(mv[:, t, :], ctx_stats[:, t])
                eps_t = tmpp.tile([128, 1], f32, tag="ln_eps", name="ln_eps")
                nc.vector.memset(eps_t[:], EPS)
                rstd = tmpp.tile([128, NT], f32, tag="crstd", name="crstd", bufs=2)
                var_ap = bass.AP(mv.tensor, mv.offset + 2 * ts[0] + 1,
                                 [[2 * NT, 128], [2, n]])
                nc.scalar.activation(rstd[:, ts[0]:ts[0] + n], var_ap, AF.Sqrt,
                                     bias=eps_t[:], scale=1.0)
                nc.vector.reciprocal(rstd[:, ts[0]:ts[0] + n],
                                     rstd[:, ts[0]:ts[0] + n])
                negmr = tmpp.tile([128, NT], f32, tag="cnegmr", name="cnegmr",
                                  bufs=2)
                mean_ap = bass.AP(mv.tensor, mv.offset + 2 * ts[0],
                                  [[2 * NT, 128], [2, n]])
                nc.vector.tensor_mul(negmr[:, ts[0]:ts[0] + n], mean_ap,
                                     rstd[:, ts[0]:ts[0] + n])
                nc.vector.tensor_scalar_mul(negmr[:, ts[0]:ts[0] + n],
                                            negmr[:, ts[0]:ts[0] + n], -1.0)
                for t in ts:
                    nc.vector.tensor_scalar_mul(dg_r[:, t, :], ident_bf[:],
                                                rstd[:, t:t + 1])
                    nc.vector.tensor_scalar_mul(dg_n[:, t, :], ident_bf[:],
                                                negmr[:, t:t + 1])

            # ---- Wo GEMM: 2 passes of 2 token-tiles; PE transposes feed it ----
            if l + 1 < L:
                h_stats = emit_stats([None] * NT, 128, H, tag="h_stats")
            for half in range(2):
                tpair = (2 * half, 2 * half + 1)
                finish_ctx_ln(list(tpair))
                acc = {}
                for t in tpair:
                    acc[t] = (ps_a.tile([128, 512], f32, tag="A", name="wo_psA"),
                              ps_a.tile([128, 512], f32, tag="A", name="wo_psB"))
                for ct3 in range(NCI // 3):
                    woc = wstream2.tile([128, 3, H], bf16, tag="wo_l",
                                        name="wo_l", bufs=2)
                    _w0 = d_wo[l]
                    wsrc = bass.AP(_w0.tensor, _w0.offset + ct3 * 3 * 128 * H,
                                   [[H, 128], [128 * H, 3], [1, H]])
                    nc.gpsimd.dma_start(woc[:], wsrc)
                    for j in range(3):
                        ct = ct3 * 3 + j
                        pt = ps_e.tile([128, 1024], f32, tag="E", name="wotr_ps")
                        for ti, t in enumerate(tpair):
                            nc.tensor.matmul(pt[:, ti * 128:(ti + 1) * 128],
                                             glu[t][:, ct * 128:(ct + 1) * 128],
                                             dg_r[:, t, :],
                                             start=True, stop=True,
                                             skip_group_check=(ti > 0))
                            nc.tensor.matmul(pt[:, ti * 128:(ti + 1) * 128],
                                             ones_bf[:], dg_n[:, t, :],
                                             start=False, stop=True,
                                             skip_group_check=True)
                        cxt = ctxtp.tile([128, 256], bf16, tag="ctxT", name="ctxT")
                        nc.vector.tensor_copy(cxt[:], pt[:, 0:256])
                        for ti, t in enumerate(tpair):
                            nc.tensor.matmul(acc[t][0][:, 0:384],
                                             cxt[:, ti * 128:(ti + 1) * 128],
                                             woc[:, j, 0:384],
                                             start=(ct == 0), stop=(ct == NCI - 1))
                            nc.tensor.matmul(acc[t][1][:, 0:384],
                                             cxt[:, ti * 128:(ti + 1) * 128],
                                             woc[:, j, 384:768],
                                             start=(ct == 0), stop=(ct == NCI - 1))
                for t in tpair:
                    nc.vector.tensor_add(x_tiles[t][:, 0:384], x_tiles[t][:, 0:384],
                                         acc[t][0][:, 0:384])
                    nc.vector.tensor_add(x_tiles[t][:, 384:768], x_tiles[t][:, 384:768],
                                         acc[t][1][:, 0:384])
                    if l + 1 < L:
                        add_stats(h_stats, t, x_tiles[t][:], 128, H)
                        nc.vector.tensor_copy(x_bf[t][:], x_tiles[t][:])

        # ---------------- output ----------------
        for t in range(NT):
            nc.sync.dma_start(d_out[t * 128:(t + 1) * 128, :], x_tiles[t][:])

    return nc


def _prepare(inputs):
    os.environ.setdefault("JAX_PLATFORMS", "cpu")
    import ml_dtypes
    import concourse.bass as bass
    import concourse.tile as tile
    import concourse.mybir as mybir
    from concourse import bacc
    from concourse.masks import make_identity

    ids = np.asarray(inputs["input_ids"])            # [S, B] int32
    amask = np.asarray(inputs["attention_mask"])     # [B,1,1,S] bool
    pidx = np.asarray(inputs["position_indices"])    # [S, S] int32 in [0,62]
    word_emb = np.asarray(inputs["word_emb"], np.float32)
    rel_emb = np.asarray(inputs["rel_emb"], np.float32)
    rel_w = np.asarray(inputs["rel_ln_w"], np.float32)
    rel_b = np.asarray(inputs["rel_ln_b"], np.float32)
    Wv = np.asarray(inputs["Wv"], np.float32)        # [L, 2I, H]
    Wqk = np.asarray(inputs["Wqk"], np.float32)      # [L, 2H, H]
    bqk = np.asarray(inputs["bqk"], np.float32)      # [L, 2H]
    Wo = np.asarray(inputs["Wo"], np.float32)        # [L, H, I]
    l_skip = np.asarray(inputs["l_skip"], np.float32)  # [L, I]

    # ---- host prep ----
    # Toeplitz diagonal table T[s] = idx[q, q + s - 511]
    T = np.zeros(W, np.int64)
    for s in range(W):
        r = s - 511
        q0 = max(0, -r)
        T[s] = pidx[q0, q0 + r]
    T = np.clip(T, 0, NJ - 1)
    S2 = np.zeros((NJ, WPAD), np.float32)
    S2[T, np.arange(W)] = 1.0                         # col s -> one-hot T[s]
    S3 = np.zeros((NJ, WPAD), np.float32)
    S3[T[::-1], np.arange(W)] = 1.0                   # col s' -> one-hot T[1022-s']

    wqkT = np.concatenate([np.transpose(Wqk, (0, 2, 1)),
                           bqk[:, None, :]], axis=1).astype(ml_dtypes.bfloat16)
    wvT = np.transpose(Wv, (0, 2, 1)).astype(ml_dtypes.bfloat16).copy()
    woT = np.transpose(Wo, (0, 2, 1)).astype(ml_dtypes.bfloat16)  # [L, 2304, 768]
    sig = 1.0 / (1.0 + np.exp(-l_skip))                       # [L, I]
    sig_rep = np.broadcast_to(sig[:, None, :], (L, 128, I)).astype(ml_dtypes.bfloat16).copy()
    s2b = S2.astype(ml_dtypes.bfloat16)
    s3b = S3.astype(ml_dtypes.bfloat16)
    biasT = np.transpose(bqk.reshape(L, 12, 128), (0, 2, 1)).copy()  # [L,128,12]

    nc = bacc.Bacc("TRN2", target_bir_lowering=False)
    _build_program(nc, mybir, bass, tile, make_identity)
    nc.compile()

    in_maps = []
    for b in range(B):
        x0 = word_emb[ids[:, b]].astype(np.float32)           # [S, H]
        mb = (-1e30 * amask[b, 0, 0, :].astype(np.float32))   # [S]
        mb_cols = mb.reshape(NT, 128).T.copy()                # [128, NT]
        in_maps.append({
            "x0": x0, "maskbias": mb_cols,
            "ident": np.eye(128, dtype=np.float32),
            "rel_emb": rel_emb,
            "rel_w": np.broadcast_to(rel_w[None, :], (NJ, H)).astype(np.float32).copy(),
            "rel_b": np.broadcast_to(rel_b[None, :], (NJ, H)).astype(np.float32).copy(),
            "s2": s2b, "s3": s3b,
            "wqkT": wqkT, "wvT": wvT, "woT": woT, "sig": sig_rep,
            "biasT": biasT,
        })

    return nc, in_maps


def kernel(**inputs):
    from concourse.bass_utils import run_bass_kernel_spmd
    nc, in_maps = _prepare(inputs)
    res = run_bass_kernel_spmd(nc, in_maps, core_ids=list(range(B)))
    LAST_RESULT[0] = res
    out = np.stack([r["out"] for r in res.results], axis=1)   # [S, B, H]
    return out.astype(np.float32)


def bench(inputs, iters=8):
    """Build once, execute repeatedly with device-resident inputs.
    Returns (min_wall_seconds_per_exec, full_output [S,B,H], times)."""
    import time as _time
    import jax
    from jax.experimental.shard_map import shard_map
    from jax.sharding import Mesh, PartitionSpec, NamedSharding
    import concourse.mybir as mybir
    from concourse import bass2jax

    nc, in_maps = _prepare(inputs)
    bass2jax.install_neuronx_cc_hook()

    partition_name = nc.partition_id_tensor.name if nc.partition_id_tensor else None
    in_names, out_names, out_avals, zero_outs = [], [], [], []
    for alloc in nc.m.functions[0].allocations:
        if not isinstance(alloc, mybir.MemoryLocationSet):
            continue
        name = alloc.memorylocations[0].name
        if alloc.kind == "ExternalInput":
            if name != partition_name:
                in_names.append(name)
        elif alloc.kind == "ExternalOutput":
            shape = tuple(alloc.tensor_shape)
            dtype = mybir.dt.np(alloc.dtype)
            out_names.append(name)
            out_avals.append(jax.core.ShapedArray(shape, dtype))
            zero_outs.append(np.zeros(shape, dtype))
    n_params = len(in_names)
    n_outs = len(out_avals)
    all_in_names = list(in_names) + list(out_names)
    if partition_name is not None:
        all_in_names.append(partition_name)

    def _body(*args):
        operands = list(args)
        if partition_name is not None:
            operands.append(bass2jax.partition_id_tensor())
        outs = bass2jax._bass_exec_p.bind(
            *operands,
            out_avals=tuple(out_avals),
            in_names=tuple(all_in_names),
            out_names=tuple(out_names),
            lowering_input_output_aliases=(),
            sim_require_finite=True,
            sim_require_nnan=True,
            nc=nc,
        )
        return tuple(outs)

    devices = jax.devices()[:B]
    mesh = Mesh(np.asarray(devices), ("core",))
    P_ = PartitionSpec("core")
    sharded = jax.jit(
        shard_map(_body, mesh=mesh, in_specs=(P_,) * (n_params + n_outs),
                  out_specs=(P_,) * n_outs, check_rep=False),
        keep_unused=True)
    concat_in = [np.concatenate([np.asarray(in_maps[c][nm]) for c in range(B)], axis=0)
                 for nm in in_names]
    concat_zeros = [np.zeros((B * z.shape[0], *z.shape[1:]), z.dtype) for z in zero_outs]
    sh = NamedSharding(mesh, P_)
    dev_in = [jax.device_put(a, sh) for a in concat_in]
    dev_zero = [jax.device_put(a, sh) for a in concat_zeros]
    outs = sharded(*dev_in, *dev_zero)
    jax.block_until_ready(outs)
    times = []
    for _ in range(iters):
        t0 = _time.perf_counter()
        o = sharded(*dev_in, *dev_zero)
        jax.block_until_ready(o)
        times.append(_time.perf_counter() - t0)
    oi = out_names.index("out")
    full = np.asarray(outs[oi]).reshape(B, S, H).transpose(1, 0, 2)
    return min(times), full.astype(np.float32), times
